# revision 16
# baseline (speedup 1.0000x reference)
"""Causal MHA (B=4, L=2048, D=1024, H=16) on 8 NeuronCores.

Sharding: core c -> (batch b = c//2, head-group g = c%2). Data-parallel over
the 4 batches, tensor-parallel over heads (8 heads per core): wq/wk/wv
column-parallel, wo row-parallel. Each core returns a partial [L, D] output;
the host sums the two head-group partials per batch and adds wo_b.

Single fused streaming kernel, no DRAM round-trips:
  A(n):  projections in fp8e4m3 DoubleRow (4x PE rate). Weights pre-scaled
         x16 on host (avoids fp8 subnormals); the x16 on Q/K is undone by
         the exp() scale (2^-11 = 1/(8*256)), the x16 on V cancels against
         the x16 ones-column in the softmax ratio.
  B(n):  per head: S.T[keys,q] = KT_h.T @ QT_h (f32r), exp on ACT with
         full-history kb blocks PAIRED into [128,2,512] psum tiles (halves
         ACT per-instruction overhead) -> pt bf16; tri-mask diag tile (DVE);
         flipped AV: avps[q, 4t, 65] += pt_blk.T @ vaug (bf16, ones col =
         denominator on the partition axis) -> tensor_scalar_mul normalize.
  T(n):  ctxn [q,512] -> ctxT [d,q] via xbar DMA-transpose.
  C(n):  out[tok,1024] = sum_c ctxT[c].T @ wo[c] (bf16), interleaved into
         B(n+1) heads; A(n+1) units interleaved into B(n) to keep PE busy.
"""

import numpy as np
import ml_dtypes

import concourse.bacc as bacc
import concourse.bass as bass
import concourse.mybir as mybir
import concourse.tile as tile
from concourse.bass_utils import run_bass_kernel_spmd

F32 = mybir.dt.float32
F32R = mybir.dt.float32r
BF16 = mybir.dt.bfloat16
F8 = mybir.dt.float8e4
DR = mybir.MatmulPerfMode.DoubleRow

B, L, D, H, DK = 4, 2048, 1024, 16, 64
HD = 8             # heads per core
GW = 512           # head-group width (8 heads * 64)
AUGW = HD * (DK + 1)   # 520: per head 64 dims + ones col (ones LAST per head)
NCH = D // 128     # 8 contraction chunks
NSL = 4            # token slices of 512
NTT = L // 128     # 16 token tiles
WS = 1.0           # V-path pre-scale (cancels in softmax ratio; 1 for bf16)
ESC = 0.125        # exp scale: 1/sqrt(DK)


def _build_nc(dbg=False):
    nc = bacc.Bacc("TRN2", target_bir_lowering=False, debug=False, num_devices=8)

    xq = nc.dram_tensor("xq", [128, NCH, L], BF16, kind="ExternalInput").ap()
    xk = nc.dram_tensor("xk", [128, NCH, L], BF16, kind="ExternalInput").ap()
    xv = nc.dram_tensor("xv", [128, NCH, L], BF16, kind="ExternalInput").ap()
    wq = nc.dram_tensor("wq", [128, NCH, GW], BF16, kind="ExternalInput").ap()
    wk = nc.dram_tensor("wk", [128, NCH, GW], BF16, kind="ExternalInput").ap()
    wv = nc.dram_tensor("wv", [128, NCH, AUGW], BF16, kind="ExternalInput").ap()
    wo = nc.dram_tensor("wo", [128, 4, D], BF16, kind="ExternalInput").ap()
    bq = nc.dram_tensor("bq", [128, 4], F32, kind="ExternalInput").ap()
    bk = nc.dram_tensor("bk", [128, 4], F32, kind="ExternalInput").ap()
    vb = nc.dram_tensor("vb", [AUGW], F32, kind="ExternalInput").ap()
    msk = nc.dram_tensor("msk", [128, 128], BF16, kind="ExternalInput").ap()
    outp = nc.dram_tensor("outp", [L, D], F32, kind="ExternalOutput").ap()
    if dbg:
        qt_dbg = nc.dram_tensor("qt_dbg", [128, 4, GW], F32,
                                kind="ExternalOutput").ap()
        kt_dbg = nc.dram_tensor("kt_dbg", [128, 4, L], F32,
                                kind="ExternalOutput").ap()
        vg_dbg = nc.dram_tensor("vg_dbg", [128, NTT, AUGW], BF16,
                                kind="ExternalOutput").ap()
        cn_dbg = nc.dram_tensor("cn_dbg", [128, NTT, GW], BF16,
                                kind="ExternalOutput").ap()
        ct_dbg = nc.dram_tensor("ct_dbg", [128, NTT, 4, 128], BF16,
                                kind="ExternalOutput").ap()

    with tile.TileContext(nc) as tc:
        with (
            tc.tile_pool(name="persist", bufs=1) as persist,
            tc.tile_pool(name="qtp", bufs=(4 if dbg else 2)) as qtp,
            tc.tile_pool(name="xqk", bufs=(2 if dbg else 4)) as xqkp,
            tc.tile_pool(name="xvp", bufs=2) as xvp,
            tc.tile_pool(name="ptp", bufs=6) as ptp,
            tc.tile_pool(name="ctxn", bufs=(16 if dbg else 8)) as ctxnp,
            tc.tile_pool(name="ctxT", bufs=(16 if dbg else 4)) as ctxTp,
            tc.tile_pool(name="rcp", bufs=4) as rcp,
            tc.tile_pool(name="outs", bufs=2) as outsp,
            tc.tile_pool(name="psS", bufs=2, space="PSUM") as psS,
            tc.tile_pool(name="psAV", bufs=1, space="PSUM") as psAV,
            tc.tile_pool(name="psA", bufs=1, space="PSUM") as psA,
            tc.tile_pool(name="psC", bufs=2, space="PSUM") as psC,
        ):
            # ---- persistent SBUF ----
            wq_s = persist.tile([128, NCH, GW], BF16, tag="wq")
            wk_s = persist.tile([128, NCH, GW], BF16, tag="wk")
            wv_s = persist.tile([128, NCH, AUGW], BF16, tag="wv")
            wo_s = persist.tile([128, 4, D], BF16, tag="wo")
            kt_s = persist.tile([128, 4, L], F32R, tag="kt")
            vaug_s = persist.tile([128, NTT, AUGW], BF16, tag="vaug")
            bq_s = persist.tile([128, 4], F32, tag="bq")
            bk_s = persist.tile([128, 4], F32, tag="bk")
            vb_s = persist.tile([128, AUGW], F32, tag="vb")
            msk_s = persist.tile([128, 128], BF16, tag="msk")

            # weight/const loads; order = DMA engine order (startup latency)
            nc.sync.dma_start(bq_s[:, :], bq[:, :])
            nc.sync.dma_start(bk_s[:, :], bk[:, :])
            nc.sync.dma_start(wq_s[:, :, :], wq[:, :, :])
            nc.sync.dma_start(wk_s[:, :, :], wk[:, :, :])

            xq_tiles = {}
            xk_tiles = {}
            xv_tiles = {}

            def issue_xin(n):
                c0, c1 = n * 512, (n + 1) * 512
                t = xqkp.tile([128, NCH, 512], BF16, tag="xqk", name=f"xq{n}")
                nc.sync.dma_start(t[:, :, :], xq[:, :, c0:c1])
                xq_tiles[n] = t
                t = xqkp.tile([128, NCH, 512], BF16, tag="xqk", name=f"xk{n}")
                nc.sync.dma_start(t[:, :, :], xk[:, :, c0:c1])
                xk_tiles[n] = t
                t = xvp.tile([128, NCH, 512], BF16, tag="xv", name=f"xv{n}")
                nc.sync.dma_start(t[:, :, :], xv[:, :, c0:c1])
                xv_tiles[n] = t

            issue_xin(0)
            nc.sync.dma_start(wv_s[:, :, :], wv[:, :, :])
            nc.sync.dma_start(msk_s[:, :], msk[:, :])
            vb_bcast = bass.AP(tensor=vb.tensor, offset=vb.offset,
                               ap=[[0, 128], [1, AUGW]])
            nc.gpsimd.dma_start(vb_s[:, :], vb_bcast)
            nc.sync.dma_start(wo_s[:, :, :], wo[:, :, :])

            qt_tiles = {}

            # ---- phase A unit generator: fp8 DoubleRow projections ----
            def a_units(n):
                qt_t = qtp.tile([128, 4, GW], F32R, tag="qt", name=f"qt{n}")
                qt_tiles[n] = qt_t

                def qk_unit(hf, g, x_of, w_s, b_s, is_q):
                    def emit():
                        x_t = x_of[n]
                        ps = psA.tile([128, 2, 256], F32, tag="pa",
                                      name=f"pa{n}_{hf}_{g}")
                        for c in range(NCH):
                            for mi in range(2):
                                # start=True zeroes the whole psum bank:
                                # only the first write into the tile sets it
                                nc.tensor.matmul(
                                    ps[:, mi, :],
                                    w_s[:, c, (2 * g + mi) * 128:
                                        (2 * g + mi + 1) * 128],
                                    x_t[:, c, hf * 256:hf * 256 + 256],
                                    start=(c == 0 and mi == 0),
                                    stop=(c == NCH - 1),
                                    skip_group_check=True)
                        for mi in range(2):
                            m = 2 * g + mi
                            if is_q:
                                nc.vector.tensor_scalar_add(
                                    qt_t[:, m, hf * 256:hf * 256 + 256],
                                    ps[:, mi, :], b_s[:, m:m + 1])
                            else:
                                nc.vector.tensor_scalar_add(
                                    kt_s[:, m, n * 512 + hf * 256:
                                         n * 512 + hf * 256 + 256],
                                    ps[:, mi, :], b_s[:, m:m + 1])
                    return emit

                def v_unit(tt, vhf):
                    def emit():
                        ps = psA.tile([128, 260], F32, tag="pa",
                                      name=f"pv{n}_{tt}_{vhf}")
                        xv_t = xv_tiles[n]
                        for c in range(NCH):
                            nc.tensor.matmul(
                                ps[:, :],
                                xv_t[:, c, tt * 128:(tt + 1) * 128],
                                wv_s[:, c, vhf * 260:(vhf + 1) * 260],
                                start=(c == 0), stop=(c == NCH - 1))
                        nc.vector.tensor_add(
                            vaug_s[:, n * 4 + tt, vhf * 260:(vhf + 1) * 260],
                            ps[:, :], vb_s[:, vhf * 260:(vhf + 1) * 260])
                    return emit

                units = []
                for hf in range(2):
                    for g in range(2):
                        units.append(qk_unit(hf, g, xq_tiles, wq_s, bq_s, True))
                    for g in range(2):
                        units.append(qk_unit(hf, g, xk_tiles, wk_s, bk_s, False))
                    for tt in (2 * hf, 2 * hf + 1):
                        for vhf in range(2):
                            units.append(v_unit(tt, vhf))
                return units

            ctxn_tiles = {}
            ctxT_tiles = {}

            # ---- phase B: one head of slice n ----
            def b_head(n, h):
                po = (h % 2) * 64
                mc = h // 2
                qt_t = qt_tiles[n]
                nkb = 4 * n + 4
                avps = psAV.tile([128, 4, DK + 1], F32, tag="av",
                                 name=f"av{n}_{h}")
                # S/exp units: full-history kb pairs, then 4 single diag blocks
                pt_of = {}   # kb -> (tile, region or None)
                sunits = [("pair", p) for p in range(2 * n)]
                sunits += [("diag", kb) for kb in range(4 * n, 4 * n + 4)]

                def emit_s(u):
                    kind, a = u
                    sp = psS.tile([128, 2, 512], F32, tag="sp",
                                  name=f"sp{n}_{h}_{kind}{a}")
                    if kind == "pair":
                        for i in range(2):
                            kb = 2 * a + i
                            # regions 0/1 are in different banks: each needs
                            # its own start=True (bank-granular zeroing)
                            nc.tensor.matmul(
                                sp[:, i, :],
                                kt_s[po:po + 64, mc, kb * 128:(kb + 1) * 128],
                                qt_t[po:po + 64, mc, :],
                                start=True, stop=True,
                                skip_group_check=True)
                        pt = ptp.tile([128, 2, 512], BF16, tag="pt",
                                      name=f"pt{n}_{h}_p{a}")
                        nc.scalar.activation(
                            pt[:, :, :], sp[:, :, :],
                            func=mybir.ActivationFunctionType.Exp, scale=ESC)
                        pt_of[2 * a] = (pt, 0)
                        pt_of[2 * a + 1] = (pt, 1)
                    else:
                        kb = a
                        jj = kb - 4 * n
                        col0e = jj * 128
                        col0s = min(col0e, 256)
                        nc.tensor.matmul(
                            sp[:, 0, col0s:],
                            kt_s[po:po + 64, mc, kb * 128:(kb + 1) * 128],
                            qt_t[po:po + 64, mc, col0s:],
                            start=True, stop=True, skip_group_check=True)
                        pt = ptp.tile([128, 512], BF16, tag="pt",
                                      name=f"pt{n}_{h}_d{jj}")
                        nc.scalar.activation(
                            pt[:, col0e:], sp[:, 0, col0e:],
                            func=mybir.ActivationFunctionType.Exp, scale=ESC)
                        nc.vector.tensor_mul(
                            pt[:, col0e:col0e + 128],
                            pt[:, col0e:col0e + 128], msk_s[:, :])
                        pt_of[kb] = (pt, None)

                def emit_av(u):
                    kind, a = u
                    kbs = [2 * a, 2 * a + 1] if kind == "pair" else [a]
                    for kb in kbs:
                        j0 = max(0, kb - 4 * n)
                        pt, reg = pt_of[kb]
                        for j in range(j0, 4):
                            lhs = (pt[:, reg, j * 128:(j + 1) * 128]
                                   if reg is not None
                                   else pt[:, j * 128:(j + 1) * 128])
                            # whole-bank zero on start: only first mm sets it
                            nc.tensor.matmul(
                                avps[:, j, :], lhs,
                                vaug_s[:, kb, h * 65:(h + 1) * 65],
                                start=(kb == 0 and j == 0),
                                stop=(kb == 4 * n + j),
                                skip_group_check=True)

                emit_s(sunits[0])
                if len(sunits) > 1:
                    emit_s(sunits[1])
                for i in range(2, len(sunits)):
                    emit_s(sunits[i])
                    emit_av(sunits[i - 2])
                emit_av(sunits[-2])
                emit_av(sunits[-1])

                rc = rcp.tile([128, 4], F32, tag="rc", name=f"rc{n}_{h}")
                nc.vector.reciprocal(rc[:, :], avps[:, :, 64])
                for j in range(4):
                    nc.vector.tensor_scalar_mul(
                        ctxn_tiles[(n, j)][:, h * 64:(h + 1) * 64],
                        avps[:, j, 0:64], rc[:, j:j + 1])

            # ---- phase C unit: token tile t, output half n2 ----
            out_tiles = {}

            def c_unit(n, j, n2):
                t = 4 * n + j

                def emit():
                    if n2 == 0:
                        out_tiles[t] = outsp.tile([128, D], F32, tag="outs",
                                                  name=f"out{t}")
                    cps = psC.tile([128, 512], F32, tag="cps",
                                   name=f"cps{t}_{n2}")
                    ctxT_t = ctxT_tiles[(n, j)]
                    for c in range(4):
                        nc.tensor.matmul(
                            cps[:, :], ctxT_t[:, c, :],
                            wo_s[:, c, n2 * 512:(n2 + 1) * 512],
                            start=(c == 0), stop=(c == 3))
                    nc.vector.tensor_copy(
                        out_tiles[t][:, n2 * 512:(n2 + 1) * 512], cps[:, :])
                    if n2 == 1:
                        nc.sync.dma_start(
                            outp[t * 128:(t + 1) * 128, :], out_tiles[t][:, :])
                return emit

            # ---- main schedule ----
            for u in a_units(0):
                u()

            pending_c = []
            for n in range(NSL):
                if n < NSL - 1:
                    issue_xin(n + 1)
                for j in range(4):
                    ctxn_tiles[(n, j)] = ctxnp.tile(
                        [128, GW], BF16, tag="ctxn", name=f"ctxn{n}_{j}")
                au = a_units(n + 1) if n < NSL - 1 else []
                ai = 0
                for h in range(HD):
                    b_head(n, h)
                    for _ in range(2):
                        if ai < len(au):
                            au[ai]()
                            ai += 1
                    if pending_c:
                        pending_c.pop(0)()
                while ai < len(au):
                    au[ai]()
                    ai += 1
                while pending_c:
                    pending_c.pop(0)()
                for j in range(4):
                    ct = ctxTp.tile([128, 4, 128], BF16, tag="ctxT",
                                    name=f"ctxT{n}_{j}")
                    nc.sync.dma_start_transpose(ct, ctxn_tiles[(n, j)][:, :])
                    ctxT_tiles[(n, j)] = ct
                for j in range(4):
                    for n2 in range(2):
                        pending_c.append(c_unit(n, j, n2))
            while pending_c:
                pending_c.pop(0)()

            if dbg:
                nc.sync.dma_start(qt_dbg[:, :, :],
                                  qt_tiles[0][:, :, :].bitcast(F32))
                nc.sync.dma_start(kt_dbg[:, :, :], kt_s[:, :, :].bitcast(F32))
                nc.sync.dma_start(vg_dbg[:, :, :], vaug_s[:, :, :])
                for n in range(NSL):
                    for j in range(4):
                        nc.sync.dma_start(cn_dbg[:, 4 * n + j, :],
                                          ctxn_tiles[(n, j)][:, :])
                        nc.sync.dma_start(ct_dbg[:, 4 * n + j, :, :],
                                          ctxT_tiles[(n, j)][:, :, :])

    nc.compile()
    return nc


_NC = None
LAST_RESULTS = None


def kernel(**inputs):
    global _NC, LAST_RESULTS
    import os
    if _NC is None:
        _NC = _build_nc()

    f = lambda a: np.asarray(a, dtype=np.float32)
    q, k, v = f(inputs["q"]), f(inputs["k"]), f(inputs["v"])
    wq_w, wq_b = f(inputs["wq_w"]), f(inputs["wq_b"])
    wk_w, wk_b = f(inputs["wk_w"]), f(inputs["wk_b"])
    wv_w, wv_b = f(inputs["wv_w"]), f(inputs["wv_b"])
    wo_w, wo_b = f(inputs["wo_w"]), f(inputs["wo_b"])

    bf = ml_dtypes.bfloat16
    f8 = ml_dtypes.float8_e4m3

    def chunk_rows(a, inner):
        # [1024, X] -> [128, 8, X] with row r = c*128+p -> [p, c, :]
        return np.ascontiguousarray(
            a.reshape(NCH, 128, inner).transpose(1, 0, 2))

    msk = np.ascontiguousarray(
        (np.arange(128)[None, :] >= np.arange(128)[:, None])).astype(bf)

    gmaps = []
    for g in range(2):
        sl = slice(g * GW, (g + 1) * GW)
        wqT = chunk_rows(wq_w[sl].T, GW).astype(bf)
        wkT = chunk_rows(wk_w[sl].T, GW).astype(bf)
        wvT = np.zeros((D, AUGW), np.float32)
        vbias = np.zeros((AUGW,), np.float32)
        for h in range(HD):
            wvT[:, h * 65:h * 65 + 64] = wv_w[g * GW + h * 64:
                                              g * GW + (h + 1) * 64].T * WS
            vbias[h * 65:h * 65 + 64] = wv_b[g * GW + h * 64:
                                             g * GW + (h + 1) * 64] * WS
            vbias[h * 65 + 64] = WS
        woT = np.ascontiguousarray(
            wo_w[:, sl].T.reshape(4, 128, D).transpose(1, 0, 2)).astype(bf)
        bqT = np.ascontiguousarray(wq_b[sl].reshape(4, 128).T)
        bkT = np.ascontiguousarray(wk_b[sl].reshape(4, 128).T)
        gmaps.append(dict(wq=wqT, wk=wkT, wv=chunk_rows(wvT, AUGW).astype(bf),
                          wo=woT, bq=bqT, bk=bkT, vb=vbias, msk=msk))

    bmaps = []
    for b in range(B):
        bmaps.append(dict(
            xq=chunk_rows(np.ascontiguousarray(q[b].T), L).astype(bf),
            xk=chunk_rows(np.ascontiguousarray(k[b].T), L).astype(bf),
            xv=chunk_rows(np.ascontiguousarray(v[b].T), L).astype(bf)))

    in_maps = [dict(**bmaps[c // 2], **gmaps[c % 2]) for c in range(8)]

    trace = bool(int(os.environ.get("KERNEL_TRACE", "0")))
    res = run_bass_kernel_spmd(_NC, in_maps, list(range(8)), trace=trace)
    LAST_RESULTS = res

    out = np.empty((B, L, D), np.float32)
    for b in range(B):
        out[b] = (res.results[2 * b]["outp"] + res.results[2 * b + 1]["outp"]
                  + wo_b[None, :])
    return out


# revision 22
# speedup vs baseline: 1.0722x; 1.0722x over previous
"""Causal MHA (B=4, L=2048, D=1024, H=16) on 8 NeuronCores.

Sharding: core c -> (batch b = c//2, head-group g = c%2). Data-parallel over
the 4 batches, tensor-parallel over heads (8 heads per core): wq/wk/wv
column-parallel, wo row-parallel. Each core returns a partial [L, D] output;
the host sums the two head-group partials per batch and adds wo_b.

Single fused streaming kernel, no DRAM round-trips:
  A(n):  projections in fp8e4m3 DoubleRow (4x PE rate). Weights pre-scaled
         x16 on host (avoids fp8 subnormals); the x16 on Q/K is undone by
         the exp() scale (2^-11 = 1/(8*256)), the x16 on V cancels against
         the x16 ones-column in the softmax ratio.
  B(n):  per head: S.T[keys,q] = KT_h.T @ QT_h (f32r), exp on ACT with
         full-history kb blocks PAIRED into [128,2,512] psum tiles (halves
         ACT per-instruction overhead) -> pt bf16; tri-mask diag tile (DVE);
         flipped AV: avps[q, 4t, 65] += pt_blk.T @ vaug (bf16, ones col =
         denominator on the partition axis) -> tensor_scalar_mul normalize.
  T(n):  ctxn [q,512] -> ctxT [d,q] via xbar DMA-transpose.
  C(n):  out[tok,1024] = sum_c ctxT[c].T @ wo[c] (bf16), interleaved into
         B(n+1) heads; A(n+1) units interleaved into B(n) to keep PE busy.
"""

import numpy as np
import ml_dtypes

import concourse.bacc as bacc
import concourse.bass as bass
import concourse.mybir as mybir
import concourse.tile as tile
from concourse.bass_utils import run_bass_kernel_spmd

F32 = mybir.dt.float32
F32R = mybir.dt.float32r
BF16 = mybir.dt.bfloat16
F8 = mybir.dt.float8e4
DR = mybir.MatmulPerfMode.DoubleRow

B, L, D, H, DK = 4, 2048, 1024, 16, 64
HD = 8             # heads per core
GW = 512           # head-group width (8 heads * 64)
AUGW = HD * (DK + 1)   # 520: per head 64 dims + ones col (ones LAST per head)
NCH = D // 128     # 8 contraction chunks
NSL = 4            # token slices of 512
NTT = L // 128     # 16 token tiles
WS = 1.0           # V-path pre-scale (cancels in softmax ratio; 1 for bf16)
ESC = 0.125        # exp scale: 1/sqrt(DK)
EBI = -2.0         # exp bias: shift-invariant headroom so exp fits fp8e4m3


def _build_nc(dbg=False):
    nc = bacc.Bacc("TRN2", target_bir_lowering=False, debug=False, num_devices=8)

    xq = nc.dram_tensor("xq", [128, NCH, L], BF16, kind="ExternalInput").ap()
    xk = nc.dram_tensor("xk", [128, NCH, L], BF16, kind="ExternalInput").ap()
    xv = nc.dram_tensor("xv", [128, NCH, L], BF16, kind="ExternalInput").ap()
    wq = nc.dram_tensor("wq", [128, NCH, GW], BF16, kind="ExternalInput").ap()
    wk = nc.dram_tensor("wk", [128, NCH, GW], BF16, kind="ExternalInput").ap()
    wv = nc.dram_tensor("wv", [128, NCH, AUGW], BF16, kind="ExternalInput").ap()
    wo = nc.dram_tensor("wo", [128, 4, D], BF16, kind="ExternalInput").ap()
    bq = nc.dram_tensor("bq", [128, 4], F32, kind="ExternalInput").ap()
    bk = nc.dram_tensor("bk", [128, 4], F32, kind="ExternalInput").ap()
    vb = nc.dram_tensor("vb", [AUGW], F32, kind="ExternalInput").ap()
    msk = nc.dram_tensor("msk", [128, 128], BF16, kind="ExternalInput").ap()
    outp = nc.dram_tensor("outp", [L, D], F32, kind="ExternalOutput").ap()
    if dbg:
        qt_dbg = nc.dram_tensor("qt_dbg", [128, 4, GW], F32,
                                kind="ExternalOutput").ap()
        kt_dbg = nc.dram_tensor("kt_dbg", [128, 4, L], F32,
                                kind="ExternalOutput").ap()
        vg_dbg = nc.dram_tensor("vg_dbg", [128, NTT, AUGW], BF16,
                                kind="ExternalOutput").ap()
        cn_dbg = nc.dram_tensor("cn_dbg", [128, NTT, GW], BF16,
                                kind="ExternalOutput").ap()
        ct_dbg = nc.dram_tensor("ct_dbg", [128, NTT, 4, 128], BF16,
                                kind="ExternalOutput").ap()

    with tile.TileContext(nc) as tc:
        with (
            tc.tile_pool(name="persist", bufs=1) as persist,
            tc.tile_pool(name="qtp", bufs=(4 if dbg else 2)) as qtp,
            tc.tile_pool(name="xqk", bufs=(2 if dbg else 4)) as xqkp,
            tc.tile_pool(name="xvp", bufs=2) as xvp,
            tc.tile_pool(name="ptp", bufs=6) as ptp,
            tc.tile_pool(name="ctxn", bufs=(16 if dbg else 8)) as ctxnp,
            tc.tile_pool(name="ctxT", bufs=(16 if dbg else 4)) as ctxTp,
            tc.tile_pool(name="rcp", bufs=4) as rcp,
            tc.tile_pool(name="outs", bufs=2) as outsp,
            tc.tile_pool(name="psS", bufs=2, space="PSUM") as psS,
            tc.tile_pool(name="psAV", bufs=1, space="PSUM") as psAV,
            tc.tile_pool(name="psA", bufs=2, space="PSUM") as psA,
            tc.tile_pool(name="psC", bufs=1, space="PSUM") as psC,
        ):
            # ---- persistent SBUF ----
            wq_s = persist.tile([128, NCH, GW], BF16, tag="wq")
            wk_s = persist.tile([128, NCH, GW], BF16, tag="wk")
            wv_s = persist.tile([128, NCH, AUGW], BF16, tag="wv")
            wo_s = persist.tile([128, 4, D], BF16, tag="wo")
            kt_s = persist.tile([128, 4, L], F32R, tag="kt")
            vaug_s = persist.tile([128, NTT, AUGW], BF16, tag="vaug")
            bq_s = persist.tile([128, 4], F32, tag="bq")
            bk_s = persist.tile([128, 4], F32, tag="bk")
            vb_s = persist.tile([128, AUGW], F32, tag="vb")
            msk_s = persist.tile([128, 128], BF16, tag="msk")

            # weight/const loads; order = DMA engine order (startup latency)
            nc.sync.dma_start(bq_s[:, :], bq[:, :])
            nc.sync.dma_start(bk_s[:, :], bk[:, :])
            vb_bcast = bass.AP(tensor=vb.tensor, offset=vb.offset,
                               ap=[[0, 128], [1, AUGW]])
            nc.gpsimd.dma_start(vb_s[:, :], vb_bcast)

            xq_tiles = {}
            xk_tiles = {}
            xv_tiles = {}

            def issue_xin(n):
                c0, c1 = n * 512, (n + 1) * 512
                t = xqkp.tile([128, NCH, 512], BF16, tag="xqk", name=f"xq{n}")
                nc.sync.dma_start(t[:, :, :], xq[:, :, c0:c1])
                xq_tiles[n] = t
                t = xqkp.tile([128, NCH, 512], BF16, tag="xqk", name=f"xk{n}")
                nc.sync.dma_start(t[:, :, :], xk[:, :, c0:c1])
                xk_tiles[n] = t
                t = xvp.tile([128, NCH, 512], BF16, tag="xv", name=f"xv{n}")
                nc.sync.dma_start(t[:, :, :], xv[:, :, c0:c1])
                xv_tiles[n] = t

            # startup order matches phase-A consumption: Q, K, then V
            nc.sync.dma_start(wq_s[:, :, 0:256], wq[:, :, 0:256])
            t0 = xqkp.tile([128, NCH, 512], BF16, tag="xqk", name="xq0")
            nc.sync.dma_start(t0[:, :, :], xq[:, :, 0:512])
            xq_tiles[0] = t0
            nc.sync.dma_start(wq_s[:, :, 256:512], wq[:, :, 256:512])
            nc.sync.dma_start(wk_s[:, :, :], wk[:, :, :])
            t0 = xqkp.tile([128, NCH, 512], BF16, tag="xqk", name="xk0")
            nc.sync.dma_start(t0[:, :, :], xk[:, :, 0:512])
            xk_tiles[0] = t0
            nc.sync.dma_start(wv_s[:, :, :], wv[:, :, :])
            t0 = xvp.tile([128, NCH, 512], BF16, tag="xv", name="xv0")
            nc.sync.dma_start(t0[:, :, :], xv[:, :, 0:512])
            xv_tiles[0] = t0
            nc.sync.dma_start(msk_s[:, :], msk[:, :])
            nc.sync.dma_start(wo_s[:, :, :], wo[:, :, :])

            qt_tiles = {}

            # ---- phase A unit generator: fp8 DoubleRow projections ----
            def a_units(n):
                qt_t = qtp.tile([128, 4, GW], F32R, tag="qt", name=f"qt{n}")
                qt_tiles[n] = qt_t

                def qk_unit(hf, g, x_of, w_s, b_s, is_q):
                    def emit():
                        x_t = x_of[n]
                        ps = psA.tile([128, 2, 256], F32, tag="pa",
                                      name=f"pa{n}_{hf}_{g}")
                        for c in range(NCH):
                            for mi in range(2):
                                # start=True zeroes the whole psum bank:
                                # only the first write into the tile sets it
                                nc.tensor.matmul(
                                    ps[:, mi, :],
                                    w_s[:, c, (2 * g + mi) * 128:
                                        (2 * g + mi + 1) * 128],
                                    x_t[:, c, hf * 256:hf * 256 + 256],
                                    start=(c == 0 and mi == 0),
                                    stop=(c == NCH - 1),
                                    skip_group_check=True)
                        for mi in range(2):
                            m = 2 * g + mi
                            if is_q:
                                nc.vector.tensor_scalar_add(
                                    qt_t[:, m, hf * 256:hf * 256 + 256],
                                    ps[:, mi, :], b_s[:, m:m + 1])
                            else:
                                nc.vector.tensor_scalar_add(
                                    kt_s[:, m, n * 512 + hf * 256:
                                         n * 512 + hf * 256 + 256],
                                    ps[:, mi, :], b_s[:, m:m + 1])
                    return emit

                def v_unit(tt, vhf):
                    def emit():
                        ps = psA.tile([128, 260], F32, tag="pa",
                                      name=f"pv{n}_{tt}_{vhf}")
                        xv_t = xv_tiles[n]
                        for c in range(NCH):
                            nc.tensor.matmul(
                                ps[:, :],
                                xv_t[:, c, tt * 128:(tt + 1) * 128],
                                wv_s[:, c, vhf * 260:(vhf + 1) * 260],
                                start=(c == 0), stop=(c == NCH - 1))
                        nc.vector.tensor_add(
                            vaug_s[:, n * 4 + tt, vhf * 260:(vhf + 1) * 260],
                            ps[:, :], vb_s[:, vhf * 260:(vhf + 1) * 260])
                    return emit

                units = []
                for hf in range(2):
                    for g in range(2):
                        units.append(qk_unit(hf, g, xq_tiles, wq_s, bq_s, True))
                    for g in range(2):
                        units.append(qk_unit(hf, g, xk_tiles, wk_s, bk_s, False))
                    for tt in (2 * hf, 2 * hf + 1):
                        for vhf in range(2):
                            units.append(v_unit(tt, vhf))
                return units

            ctxn_tiles = {}
            ctxT_tiles = {}

            # ---- phase B: one head of slice n ----
            def b_head(n, h):
                po = (h % 2) * 64
                mc = h // 2
                qt_t = qt_tiles[n]
                nkb = 4 * n + 4
                avps = psAV.tile([128, 4, DK + 1], F32, tag="av",
                                 name=f"av{n}_{h}")
                # S/exp units: full-history kb pairs, then 4 single diag blocks
                pt_of = {}   # kb -> (tile, region or None)
                sunits = [("pair", p) for p in range(2 * n)]
                sunits += [("diag", kb) for kb in range(4 * n, 4 * n + 4)]

                def emit_s(u):
                    kind, a = u
                    sp = psS.tile([128, 2, 512], F32, tag="sp",
                                  name=f"sp{n}_{h}_{kind}{a}")
                    if kind == "pair":
                        for i in range(2):
                            kb = 2 * a + i
                            # regions 0/1 are in different banks: each needs
                            # its own start=True (bank-granular zeroing)
                            nc.tensor.matmul(
                                sp[:, i, :],
                                kt_s[po:po + 64, mc, kb * 128:(kb + 1) * 128],
                                qt_t[po:po + 64, mc, :],
                                start=True, stop=True,
                                skip_group_check=True)
                        pt = ptp.tile([128, 2, 512], BF16, tag="pt",
                                      name=f"pt{n}_{h}_p{a}")
                        nc.scalar.activation(
                            pt[:, :, :], sp[:, :, :],
                            func=mybir.ActivationFunctionType.Exp, scale=ESC)
                        pt_of[2 * a] = (pt, 0)
                        pt_of[2 * a + 1] = (pt, 1)
                    else:
                        kb = a
                        jj = kb - 4 * n
                        col0e = jj * 128
                        col0s = min(col0e, 256)
                        nc.tensor.matmul(
                            sp[:, 0, col0s:],
                            kt_s[po:po + 64, mc, kb * 128:(kb + 1) * 128],
                            qt_t[po:po + 64, mc, col0s:],
                            start=True, stop=True, skip_group_check=True)
                        pt = ptp.tile([128, 512], BF16, tag="pt",
                                      name=f"pt{n}_{h}_d{jj}")
                        nc.scalar.activation(
                            pt[:, col0e:], sp[:, 0, col0e:],
                            func=mybir.ActivationFunctionType.Exp, scale=ESC)
                        nc.vector.tensor_mul(
                            pt[:, col0e:col0e + 128],
                            pt[:, col0e:col0e + 128], msk_s[:, :])
                        pt_of[kb] = (pt, None)

                def emit_av(u):
                    kind, a = u
                    kbs = [2 * a, 2 * a + 1] if kind == "pair" else [a]
                    for kb in kbs:
                        j0 = max(0, kb - 4 * n)
                        pt, reg = pt_of[kb]
                        for j in range(j0, 4):
                            lhs = (pt[:, reg, j * 128:(j + 1) * 128]
                                   if reg is not None
                                   else pt[:, j * 128:(j + 1) * 128])
                            # whole-bank zero on start: only first mm sets it
                            nc.tensor.matmul(
                                avps[:, j, :], lhs,
                                vaug_s[:, kb, h * 65:(h + 1) * 65],
                                start=(kb == 0 and j == 0),
                                stop=(kb == 4 * n + j),
                                skip_group_check=True)

                emit_s(sunits[0])
                if len(sunits) > 1:
                    emit_s(sunits[1])
                for i in range(2, len(sunits)):
                    emit_s(sunits[i])
                    emit_av(sunits[i - 2])
                emit_av(sunits[-2])
                emit_av(sunits[-1])

                rc = rcp.tile([128, 4], F32, tag="rc", name=f"rc{n}_{h}")
                nc.vector.reciprocal(rc[:, :], avps[:, :, 64])
                for j in range(4):
                    nc.vector.tensor_scalar_mul(
                        ctxn_tiles[(n, j)][:, h * 64:(h + 1) * 64],
                        avps[:, j, 0:64], rc[:, j:j + 1])

            # ---- phase C unit: token tile t, output half n2 ----
            out_tiles = {}

            def c_unit(n, j, n2):
                t = 4 * n + j
                # during B(3) no A-units run: C for slices 2-3 alternates
                # between psC and the idle psA pool for 2-deep pipelining
                pool = psA if (n >= 2 and (2 * j + n2) % 2 == 1) else psC
                ptag = "pa" if pool is psA else "cps"

                def emit():
                    if n2 == 0:
                        out_tiles[t] = outsp.tile([128, D], F32, tag="outs",
                                                  name=f"out{t}")
                    cps = pool.tile([128, 512], F32, tag=ptag,
                                    name=f"cps{t}_{n2}")
                    ctxT_t = ctxT_tiles[(n, j)]
                    for c in range(4):
                        nc.tensor.matmul(
                            cps[:, :], ctxT_t[:, c, :],
                            wo_s[:, c, n2 * 512:(n2 + 1) * 512],
                            start=(c == 0), stop=(c == 3))
                    nc.vector.tensor_copy(
                        out_tiles[t][:, n2 * 512:(n2 + 1) * 512], cps[:, :])
                    if n2 == 1:
                        nc.sync.dma_start(
                            outp[t * 128:(t + 1) * 128, :], out_tiles[t][:, :])
                return emit

            # ---- main schedule ----
            for u in a_units(0):
                u()

            pending_c = []
            for n in range(NSL):
                if n < NSL - 1:
                    issue_xin(n + 1)
                for j in range(4):
                    ctxn_tiles[(n, j)] = ctxnp.tile(
                        [128, GW], BF16, tag="ctxn", name=f"ctxn{n}_{j}")
                au = a_units(n + 1) if n < NSL - 1 else []
                ai = 0
                for h in range(HD):
                    b_head(n, h)
                    for _ in range(2):
                        if ai < len(au):
                            au[ai]()
                            ai += 1
                    for _ in range(2):
                        if pending_c:
                            pending_c.pop(0)()
                while ai < len(au):
                    au[ai]()
                    ai += 1
                while pending_c:
                    pending_c.pop(0)()
                for j in range(4):
                    ct = ctxTp.tile([128, 4, 128], BF16, tag="ctxT",
                                    name=f"ctxT{n}_{j}")
                    nc.sync.dma_start_transpose(ct, ctxn_tiles[(n, j)][:, :])
                    ctxT_tiles[(n, j)] = ct
                for j in range(4):
                    for n2 in range(2):
                        pending_c.append(c_unit(n, j, n2))
            while pending_c:
                pending_c.pop(0)()

            if dbg:
                nc.sync.dma_start(qt_dbg[:, :, :],
                                  qt_tiles[0][:, :, :].bitcast(F32))
                nc.sync.dma_start(kt_dbg[:, :, :], kt_s[:, :, :].bitcast(F32))
                nc.sync.dma_start(vg_dbg[:, :, :], vaug_s[:, :, :])
                for n in range(NSL):
                    for j in range(4):
                        nc.sync.dma_start(cn_dbg[:, 4 * n + j, :],
                                          ctxn_tiles[(n, j)][:, :])
                        nc.sync.dma_start(ct_dbg[:, 4 * n + j, :, :],
                                          ctxT_tiles[(n, j)][:, :, :])

    nc.compile()
    return nc


_NC = None
LAST_RESULTS = None


def kernel(**inputs):
    global _NC, LAST_RESULTS
    import os
    if _NC is None:
        _NC = _build_nc()

    f = lambda a: np.asarray(a, dtype=np.float32)
    q, k, v = f(inputs["q"]), f(inputs["k"]), f(inputs["v"])
    wq_w, wq_b = f(inputs["wq_w"]), f(inputs["wq_b"])
    wk_w, wk_b = f(inputs["wk_w"]), f(inputs["wk_b"])
    wv_w, wv_b = f(inputs["wv_w"]), f(inputs["wv_b"])
    wo_w, wo_b = f(inputs["wo_w"]), f(inputs["wo_b"])

    bf = ml_dtypes.bfloat16
    f8 = ml_dtypes.float8_e4m3

    def chunk_rows(a, inner):
        # [1024, X] -> [128, 8, X] with row r = c*128+p -> [p, c, :]
        return np.ascontiguousarray(
            a.reshape(NCH, 128, inner).transpose(1, 0, 2))

    msk = np.ascontiguousarray(
        (np.arange(128)[None, :] >= np.arange(128)[:, None])).astype(bf)

    gmaps = []
    for g in range(2):
        sl = slice(g * GW, (g + 1) * GW)
        wqT = chunk_rows(wq_w[sl].T, GW).astype(bf)
        wkT = chunk_rows(wk_w[sl].T, GW).astype(bf)
        wvT = np.zeros((D, AUGW), np.float32)
        vbias = np.zeros((AUGW,), np.float32)
        for h in range(HD):
            wvT[:, h * 65:h * 65 + 64] = wv_w[g * GW + h * 64:
                                              g * GW + (h + 1) * 64].T * WS
            vbias[h * 65:h * 65 + 64] = wv_b[g * GW + h * 64:
                                             g * GW + (h + 1) * 64] * WS
            vbias[h * 65 + 64] = WS
        woT = np.ascontiguousarray(
            wo_w[:, sl].T.reshape(4, 128, D).transpose(1, 0, 2)).astype(bf)
        bqT = np.ascontiguousarray(wq_b[sl].reshape(4, 128).T)
        bkT = np.ascontiguousarray(wk_b[sl].reshape(4, 128).T)
        gmaps.append(dict(wq=wqT, wk=wkT, wv=chunk_rows(wvT, AUGW).astype(bf),
                          wo=woT, bq=bqT, bk=bkT, vb=vbias, msk=msk))

    bmaps = []
    for b in range(B):
        bmaps.append(dict(
            xq=chunk_rows(np.ascontiguousarray(q[b].T), L).astype(bf),
            xk=chunk_rows(np.ascontiguousarray(k[b].T), L).astype(bf),
            xv=chunk_rows(np.ascontiguousarray(v[b].T), L).astype(bf)))

    in_maps = [dict(**bmaps[c // 2], **gmaps[c % 2]) for c in range(8)]

    trace = bool(int(os.environ.get("KERNEL_TRACE", "0")))
    res = run_bass_kernel_spmd(_NC, in_maps, list(range(8)), trace=trace)
    LAST_RESULTS = res

    out = np.empty((B, L, D), np.float32)
    for b in range(B):
        out[b] = (res.results[2 * b]["outp"] + res.results[2 * b + 1]["outp"]
                  + wo_b[None, :])
    return out


# revision 29
# speedup vs baseline: 1.0792x; 1.0065x over previous
"""Causal MHA (B=4, L=2048, D=1024, H=16) on 8 NeuronCores.

Sharding: core c -> (batch b = c//2, head-group g = c%2). Data-parallel over
the 4 batches, tensor-parallel over heads (8 heads per core): wq/wk/wv
column-parallel, wo row-parallel. Each core returns a partial [L, D] output;
the host sums the two head-group partials per batch and adds wo_b.

Single fused streaming kernel, no DRAM round-trips:
  A(n):  projections in fp8e4m3 DoubleRow (4x PE rate). Weights pre-scaled
         x16 on host (avoids fp8 subnormals); the x16 on Q/K is undone by
         the exp() scale (2^-11 = 1/(8*256)), the x16 on V cancels against
         the x16 ones-column in the softmax ratio.
  B(n):  per head: S.T[keys,q] = KT_h.T @ QT_h (f32r), exp on ACT with
         full-history kb blocks PAIRED into [128,2,512] psum tiles (halves
         ACT per-instruction overhead) -> pt bf16; tri-mask diag tile (DVE);
         flipped AV: avps[q, 4t, 65] += pt_blk.T @ vaug (bf16, ones col =
         denominator on the partition axis) -> tensor_scalar_mul normalize.
  T(n):  ctxn [q,512] -> ctxT [d,q] via xbar DMA-transpose.
  C(n):  out[tok,1024] = sum_c ctxT[c].T @ wo[c] (bf16), interleaved into
         B(n+1) heads; A(n+1) units interleaved into B(n) to keep PE busy.
"""

import numpy as np
import ml_dtypes

import concourse.bacc as bacc
import concourse.bass as bass
import concourse.mybir as mybir
import concourse.tile as tile
from concourse.bass_utils import run_bass_kernel_spmd

F32 = mybir.dt.float32
F32R = mybir.dt.float32r
BF16 = mybir.dt.bfloat16
F8 = mybir.dt.float8e4
DR = mybir.MatmulPerfMode.DoubleRow

B, L, D, H, DK = 4, 2048, 1024, 16, 64
HD = 8             # heads per core
GW = 512           # head-group width (8 heads * 64)
AUGW = HD * (DK + 1)   # 520: per head 64 dims + ones col (ones LAST per head)
NCH = D // 128     # 8 contraction chunks
NSL = 4            # token slices of 512
NTT = L // 128     # 16 token tiles
WS = 1.0           # V-path pre-scale (cancels in softmax ratio; 1 for bf16)
ESC = 0.125        # exp scale: 1/sqrt(DK)
EBI = -2.0         # exp bias: shift-invariant headroom so exp fits fp8e4m3


def _build_nc(dbg=False):
    import os
    LOOKAHEAD = bool(int(os.environ.get("KCFG_LOOKAHEAD", "0")))
    ARATE = int(os.environ.get("KCFG_ARATE", "2"))
    CRATES = [int(x) for x in os.environ.get("KCFG_CRATE", "2,2,2,2").split(",")]
    INHEAD = bool(int(os.environ.get("KCFG_INHEAD", "0")))
    BIASACT = bool(int(os.environ.get("KCFG_BIASACT", "0")))
    nc = bacc.Bacc("TRN2", target_bir_lowering=False, debug=False, num_devices=8)

    xq = nc.dram_tensor("xq", [128, NCH, L], BF16, kind="ExternalInput").ap()
    xk = nc.dram_tensor("xk", [128, NCH, L], BF16, kind="ExternalInput").ap()
    xv = nc.dram_tensor("xv", [128, NCH, L], BF16, kind="ExternalInput").ap()
    wq = nc.dram_tensor("wq", [128, NCH, GW], BF16, kind="ExternalInput").ap()
    wk = nc.dram_tensor("wk", [128, NCH, GW], BF16, kind="ExternalInput").ap()
    wv = nc.dram_tensor("wv", [128, NCH, AUGW], BF16, kind="ExternalInput").ap()
    wo = nc.dram_tensor("wo", [128, 4, D], BF16, kind="ExternalInput").ap()
    bq = nc.dram_tensor("bq", [128, 4], F32, kind="ExternalInput").ap()
    bk = nc.dram_tensor("bk", [128, 4], F32, kind="ExternalInput").ap()
    vb = nc.dram_tensor("vb", [AUGW], F32, kind="ExternalInput").ap()
    msk = nc.dram_tensor("msk", [128, 128], BF16, kind="ExternalInput").ap()
    outp = nc.dram_tensor("outp", [L, D], F32, kind="ExternalOutput").ap()
    if dbg:
        qt_dbg = nc.dram_tensor("qt_dbg", [128, 4, GW], F32,
                                kind="ExternalOutput").ap()
        kt_dbg = nc.dram_tensor("kt_dbg", [128, 4, L], F32,
                                kind="ExternalOutput").ap()
        vg_dbg = nc.dram_tensor("vg_dbg", [128, NTT, AUGW], BF16,
                                kind="ExternalOutput").ap()
        cn_dbg = nc.dram_tensor("cn_dbg", [128, NTT, GW], BF16,
                                kind="ExternalOutput").ap()
        ct_dbg = nc.dram_tensor("ct_dbg", [128, NTT, 4, 128], BF16,
                                kind="ExternalOutput").ap()

    with tile.TileContext(nc) as tc:
        with (
            tc.tile_pool(name="persist", bufs=1) as persist,
            tc.tile_pool(name="qtp", bufs=(4 if dbg else 3)) as qtp,
            tc.tile_pool(name="xqk", bufs=4) as xqkp,
            tc.tile_pool(name="xvp", bufs=3) as xvp,
            tc.tile_pool(name="ptp", bufs=(4 if dbg else 5)) as ptp,
            tc.tile_pool(name="ctxn", bufs=(16 if dbg else 6)) as ctxnp,
            tc.tile_pool(name="ctxT", bufs=16) as ctxTp,
            tc.tile_pool(name="rcp", bufs=4) as rcp,
            tc.tile_pool(name="outs", bufs=2) as outsp,
            tc.tile_pool(name="psS", bufs=2, space="PSUM") as psS,
            tc.tile_pool(name="psAV", bufs=1, space="PSUM") as psAV,
            tc.tile_pool(name="psA", bufs=2, space="PSUM") as psA,
            tc.tile_pool(name="psC", bufs=1, space="PSUM") as psC,
        ):
            # ---- persistent SBUF ----
            wq_s = persist.tile([128, NCH, GW], BF16, tag="wq")
            wk_s = persist.tile([128, NCH, GW], BF16, tag="wk")
            wv_s = persist.tile([128, NCH, AUGW], BF16, tag="wv")
            wo_s = persist.tile([128, 4, D], BF16, tag="wo")
            kt_s = persist.tile([128, 4, L], F32R, tag="kt")
            vaug_s = persist.tile([128, NTT, AUGW], BF16, tag="vaug")
            bq_s = persist.tile([128, 4], F32, tag="bq")
            bk_s = persist.tile([128, 4], F32, tag="bk")
            vb_s = persist.tile([128, AUGW], BF16, tag="vb")
            msk_s = persist.tile([128, 128], BF16, tag="msk")

            # weight/const loads; order = DMA engine order (startup latency)
            nc.sync.dma_start(bq_s[:, :], bq[:, :])
            nc.sync.dma_start(bk_s[:, :], bk[:, :])
            vb_bcast = bass.AP(tensor=vb.tensor, offset=vb.offset,
                               ap=[[0, 128], [1, AUGW]])
            nc.gpsimd.dma_start(vb_s[:, :], vb_bcast)

            xq_tiles = {}
            xk_tiles = {}
            xv_tiles = {}

            def issue_xin(n):
                c0, c1 = n * 512, (n + 1) * 512
                t = xqkp.tile([128, NCH, 512], BF16, tag="xqk", name=f"xq{n}")
                nc.sync.dma_start(t[:, :, :], xq[:, :, c0:c1])
                xq_tiles[n] = t
                t = xqkp.tile([128, NCH, 512], BF16, tag="xqk", name=f"xk{n}")
                nc.sync.dma_start(t[:, :, :], xk[:, :, c0:c1])
                xk_tiles[n] = t
                t = xvp.tile([128, NCH, 512], BF16, tag="xv", name=f"xv{n}")
                nc.sync.dma_start(t[:, :, :], xv[:, :, c0:c1])
                xv_tiles[n] = t

            # startup order matches phase-A consumption: Q, K, then V
            nc.sync.dma_start(wq_s[:, :, 0:256], wq[:, :, 0:256])
            t0 = xqkp.tile([128, NCH, 512], BF16, tag="xqk", name="xq0")
            nc.sync.dma_start(t0[:, :, 0:256], xq[:, :, 0:256])
            nc.sync.dma_start(wq_s[:, :, 256:512], wq[:, :, 256:512])
            nc.sync.dma_start(t0[:, :, 256:512], xq[:, :, 256:512])
            xq_tiles[0] = t0
            nc.sync.dma_start(wk_s[:, :, :], wk[:, :, :])
            t0 = xqkp.tile([128, NCH, 512], BF16, tag="xqk", name="xk0")
            nc.sync.dma_start(t0[:, :, :], xk[:, :, 0:512])
            xk_tiles[0] = t0
            nc.sync.dma_start(wv_s[:, :, :], wv[:, :, :])
            t0 = xvp.tile([128, NCH, 512], BF16, tag="xv", name="xv0")
            nc.sync.dma_start(t0[:, :, :], xv[:, :, 0:512])
            xv_tiles[0] = t0
            nc.sync.dma_start(msk_s[:, :], msk[:, :])
            nc.sync.dma_start(wo_s[:, :, :], wo[:, :, :])

            qt_tiles = {}

            # ---- phase A unit generator: fp8 DoubleRow projections ----
            def a_units(n):
                qt_t = qtp.tile([128, 4, GW], F32R, tag="qt", name=f"qt{n}")
                qt_tiles[n] = qt_t

                def qk_unit(hf, g, x_of, w_s, b_s, is_q):
                    def emit():
                        x_t = x_of[n]
                        ps = psA.tile([128, 2, 256], F32, tag="pa",
                                      name=f"pa{n}_{hf}_{g}")
                        for c in range(NCH):
                            for mi in range(2):
                                # start=True zeroes the whole psum bank:
                                # only the first write into the tile sets it
                                nc.tensor.matmul(
                                    ps[:, mi, :],
                                    w_s[:, c, (2 * g + mi) * 128:
                                        (2 * g + mi + 1) * 128],
                                    x_t[:, c, hf * 256:hf * 256 + 256],
                                    start=(c == 0 and mi == 0),
                                    stop=(c == NCH - 1),
                                    skip_group_check=True)
                        for mi in range(2):
                            m = 2 * g + mi
                            # bias is per-partition here: use ACT (idle in
                            # the early, projection-heavy region) so DVE
                            # doesn't serialize the psA ring
                            if is_q:
                                dst = qt_t[:, m, hf * 256:hf * 256 + 256]
                            else:
                                dst = kt_s[:, m, n * 512 + hf * 256:
                                           n * 512 + hf * 256 + 256]
                            if BIASACT:
                                nc.scalar.activation(
                                    dst, ps[:, mi, :],
                                    func=mybir.ActivationFunctionType.Identity,
                                    bias=b_s[:, m:m + 1])
                            else:
                                nc.vector.tensor_scalar_add(
                                    dst, ps[:, mi, :], b_s[:, m:m + 1])
                    return emit

                def v_unit(tt, vhf):
                    def emit():
                        ps = psA.tile([128, 260], F32, tag="pa",
                                      name=f"pv{n}_{tt}_{vhf}")
                        xv_t = xv_tiles[n]
                        for c in range(NCH):
                            nc.tensor.matmul(
                                ps[:, :],
                                xv_t[:, c, tt * 128:(tt + 1) * 128],
                                wv_s[:, c, vhf * 260:(vhf + 1) * 260],
                                start=(c == 0), stop=(c == NCH - 1))
                        nc.vector.tensor_add(
                            vaug_s[:, n * 4 + tt, vhf * 260:(vhf + 1) * 260],
                            ps[:, :], vb_s[:, vhf * 260:(vhf + 1) * 260])
                    return emit

                units = []
                for hf in range(2):
                    for g in range(2):
                        units.append(qk_unit(hf, g, xq_tiles, wq_s, bq_s, True))
                    for g in range(2):
                        units.append(qk_unit(hf, g, xk_tiles, wk_s, bk_s, False))
                    for tt in (2 * hf, 2 * hf + 1):
                        for vhf in range(2):
                            units.append(v_unit(tt, vhf))
                return units

            ctxn_tiles = {}
            ctxT_tiles = {}

            # ---- phase B: one head of slice n ----
            # fillers: unit closures (A projections / C output tiles) emitted
            # inside the kb loop so they fill PE while ACT runs exp, without
            # delaying the next head's S blocks
            def b_head(n, h, fillers=()):
                po = (h % 2) * 64
                mc = h // 2
                qt_t = qt_tiles[n]
                nkb = 4 * n + 4
                avps = psAV.tile([128, 4, DK + 1], F32, tag="av",
                                 name=f"av{n}_{h}")
                # S/exp units: full-history kb pairs, then 4 single diag blocks
                pt_of = {}   # kb -> (tile, region or None)
                sunits = [("pair", p) for p in range(2 * n)]
                sunits += [("diag", kb) for kb in range(4 * n, 4 * n + 4)]

                def emit_s(u):
                    kind, a = u
                    sp = psS.tile([128, 2, 512], F32, tag="sp",
                                  name=f"sp{n}_{h}_{kind}{a}")
                    if kind == "pair":
                        for i in range(2):
                            kb = 2 * a + i
                            # regions 0/1 are in different banks: each needs
                            # its own start=True (bank-granular zeroing)
                            nc.tensor.matmul(
                                sp[:, i, :],
                                kt_s[po:po + 64, mc, kb * 128:(kb + 1) * 128],
                                qt_t[po:po + 64, mc, :],
                                start=True, stop=True,
                                skip_group_check=True)
                        pt = ptp.tile([128, 2, 512], BF16, tag="pt",
                                      name=f"pt{n}_{h}_p{a}")
                        nc.scalar.activation(
                            pt[:, :, :], sp[:, :, :],
                            func=mybir.ActivationFunctionType.Exp, scale=ESC)
                        pt_of[2 * a] = (pt, 0)
                        pt_of[2 * a + 1] = (pt, 1)
                    else:
                        kb = a
                        jj = kb - 4 * n
                        col0e = jj * 128
                        col0s = min(col0e, 256)
                        nc.tensor.matmul(
                            sp[:, 0, col0s:],
                            kt_s[po:po + 64, mc, kb * 128:(kb + 1) * 128],
                            qt_t[po:po + 64, mc, col0s:],
                            start=True, stop=True, skip_group_check=True)
                        pt = ptp.tile([128, 512], BF16, tag="pt",
                                      name=f"pt{n}_{h}_d{jj}")
                        nc.scalar.activation(
                            pt[:, col0e:], sp[:, 0, col0e:],
                            func=mybir.ActivationFunctionType.Exp, scale=ESC)
                        nc.vector.tensor_mul(
                            pt[:, col0e:col0e + 128],
                            pt[:, col0e:col0e + 128], msk_s[:, :])
                        pt_of[kb] = (pt, None)

                def emit_av(u):
                    kind, a = u
                    kbs = [2 * a, 2 * a + 1] if kind == "pair" else [a]
                    for kb in kbs:
                        j0 = max(0, kb - 4 * n)
                        pt, reg = pt_of[kb]
                        for j in range(j0, 4):
                            lhs = (pt[:, reg, j * 128:(j + 1) * 128]
                                   if reg is not None
                                   else pt[:, j * 128:(j + 1) * 128])
                            # whole-bank zero on start: only first mm sets it
                            nc.tensor.matmul(
                                avps[:, j, :], lhs,
                                vaug_s[:, kb, h * 65:(h + 1) * 65],
                                start=(kb == 0 and j == 0),
                                stop=(kb == 4 * n + j),
                                skip_group_check=True)

                fillers = list(fillers)
                nf = len(fillers)
                # spread fillers across the S-unit stream (after position 1)
                emit_s(sunits[0])
                if len(sunits) > 1:
                    emit_s(sunits[1])
                slots = max(1, len(sunits) - 1)
                fi = 0
                done = 0
                for i in range(2, len(sunits)):
                    while fi < nf and fi * slots < nf * (i - 1):
                        fillers[fi]()
                        fi += 1
                    emit_s(sunits[i])
                    emit_av(sunits[i - 2])
                while fi < nf:
                    fillers[fi]()
                    fi += 1
                emit_av(sunits[-2])
                emit_av(sunits[-1])

                rc = rcp.tile([128, 4], F32, tag="rc", name=f"rc{n}_{h}")
                nc.vector.reciprocal(rc[:, :], avps[:, :, 64])
                for j in range(4):
                    nc.vector.tensor_scalar_mul(
                        ctxn_tiles[(n, j)][:, h * 64:(h + 1) * 64],
                        avps[:, j, 0:64], rc[:, j:j + 1])

            # ---- phase C unit: token tile t, output half n2 ----
            out_tiles = {}

            def c_unit(n, j, n2):
                t = 4 * n + j
                # all C units run during B(3)/tail where psA is idle:
                # alternate psC/psA slots for deeper pipelining
                pool = psA if (2 * j + n2) % 2 == 1 else psC
                ptag = "pa" if pool is psA else "cps"

                def emit():
                    if n2 == 0:
                        out_tiles[t] = outsp.tile([128, D], F32, tag="outs",
                                                  name=f"out{t}")
                    cps = pool.tile([128, 512], F32, tag=ptag,
                                    name=f"cps{t}_{n2}")
                    ctxT_t = ctxT_tiles[(n, j)]
                    for c in range(4):
                        nc.tensor.matmul(
                            cps[:, :], ctxT_t[:, c, :],
                            wo_s[:, c, n2 * 512:(n2 + 1) * 512],
                            start=(c == 0), stop=(c == 3))
                    nc.vector.tensor_copy(
                        out_tiles[t][:, n2 * 512:(n2 + 1) * 512], cps[:, :])
                    if n2 == 1:
                        nc.sync.dma_start(
                            outp[t * 128:(t + 1) * 128, :], out_tiles[t][:, :])
                return emit

            # ---- main schedule ----
            for u in a_units(0):
                u()

            a_queue = []          # (slice, unit) in slice order
            pending_c = []
            c_rate = dict(enumerate(CRATES))

            for n in range(NSL):
                if LOOKAHEAD:
                    if n == 0:
                        issue_xin(1)
                        issue_xin(2)
                        for u in a_units(1):
                            a_queue.append((1, u))
                        for u in a_units(2):
                            a_queue.append((2, u))
                    elif n == 1:
                        issue_xin(3)
                        for u in a_units(3):
                            a_queue.append((3, u))
                else:
                    if n < NSL - 1:
                        issue_xin(n + 1)
                        for u in a_units(n + 1):
                            a_queue.append((n + 1, u))
                for j in range(4):
                    ctxn_tiles[(n, j)] = ctxnp.tile(
                        [128, GW], BF16, tag="ctxn", name=f"ctxn{n}_{j}")
                for h in range(HD):
                    fillers = []
                    for _ in range(ARATE):
                        if a_queue:
                            fillers.append(a_queue.pop(0)[1])
                    for _ in range(c_rate.get(n, 2)):
                        if pending_c:
                            fillers.append(pending_c.pop(0))
                    if INHEAD:
                        b_head(n, h, fillers)
                    else:
                        b_head(n, h)
                        for f in fillers:
                            f()
                # B(n+1) needs all of A(n+1) done
                while a_queue and a_queue[0][0] <= n + 1:
                    a_queue.pop(0)[1]()
                for j in range(4):
                    ct = ctxTp.tile([128, 4, 128], BF16, tag="ctxT",
                                    name=f"ctxT{n}_{j}")
                    nc.sync.dma_start_transpose(ct, ctxn_tiles[(n, j)][:, :])
                    ctxT_tiles[(n, j)] = ct
                for j in range(4):
                    for n2 in range(2):
                        pending_c.append(c_unit(n, j, n2))
            while pending_c:
                pending_c.pop(0)()

            if dbg:
                nc.sync.dma_start(qt_dbg[:, :, :],
                                  qt_tiles[0][:, :, :].bitcast(F32))
                nc.sync.dma_start(kt_dbg[:, :, :], kt_s[:, :, :].bitcast(F32))
                nc.sync.dma_start(vg_dbg[:, :, :], vaug_s[:, :, :])
                for n in range(NSL):
                    for j in range(4):
                        nc.sync.dma_start(cn_dbg[:, 4 * n + j, :],
                                          ctxn_tiles[(n, j)][:, :])
                        nc.sync.dma_start(ct_dbg[:, 4 * n + j, :, :],
                                          ctxT_tiles[(n, j)][:, :, :])

    nc.compile()
    return nc


_NC = None
LAST_RESULTS = None


def kernel(**inputs):
    global _NC, LAST_RESULTS
    import os
    if _NC is None:
        _NC = _build_nc()

    f = lambda a: np.asarray(a, dtype=np.float32)
    q, k, v = f(inputs["q"]), f(inputs["k"]), f(inputs["v"])
    wq_w, wq_b = f(inputs["wq_w"]), f(inputs["wq_b"])
    wk_w, wk_b = f(inputs["wk_w"]), f(inputs["wk_b"])
    wv_w, wv_b = f(inputs["wv_w"]), f(inputs["wv_b"])
    wo_w, wo_b = f(inputs["wo_w"]), f(inputs["wo_b"])

    bf = ml_dtypes.bfloat16
    f8 = ml_dtypes.float8_e4m3

    def chunk_rows(a, inner):
        # [1024, X] -> [128, 8, X] with row r = c*128+p -> [p, c, :]
        return np.ascontiguousarray(
            a.reshape(NCH, 128, inner).transpose(1, 0, 2))

    msk = np.ascontiguousarray(
        (np.arange(128)[None, :] >= np.arange(128)[:, None])).astype(bf)

    gmaps = []
    for g in range(2):
        sl = slice(g * GW, (g + 1) * GW)
        wqT = chunk_rows(wq_w[sl].T, GW).astype(bf)
        wkT = chunk_rows(wk_w[sl].T, GW).astype(bf)
        wvT = np.zeros((D, AUGW), np.float32)
        vbias = np.zeros((AUGW,), np.float32)
        for h in range(HD):
            wvT[:, h * 65:h * 65 + 64] = wv_w[g * GW + h * 64:
                                              g * GW + (h + 1) * 64].T * WS
            vbias[h * 65:h * 65 + 64] = wv_b[g * GW + h * 64:
                                             g * GW + (h + 1) * 64] * WS
            vbias[h * 65 + 64] = WS
        woT = np.ascontiguousarray(
            wo_w[:, sl].T.reshape(4, 128, D).transpose(1, 0, 2)).astype(bf)
        bqT = np.ascontiguousarray(wq_b[sl].reshape(4, 128).T)
        bkT = np.ascontiguousarray(wk_b[sl].reshape(4, 128).T)
        gmaps.append(dict(wq=wqT, wk=wkT, wv=chunk_rows(wvT, AUGW).astype(bf),
                          wo=woT, bq=bqT, bk=bkT, vb=vbias, msk=msk))

    bmaps = []
    for b in range(B):
        bmaps.append(dict(
            xq=chunk_rows(np.ascontiguousarray(q[b].T), L).astype(bf),
            xk=chunk_rows(np.ascontiguousarray(k[b].T), L).astype(bf),
            xv=chunk_rows(np.ascontiguousarray(v[b].T), L).astype(bf)))

    in_maps = [dict(**bmaps[c // 2], **gmaps[c % 2]) for c in range(8)]

    trace = bool(int(os.environ.get("KERNEL_TRACE", "0")))
    res = run_bass_kernel_spmd(_NC, in_maps, list(range(8)), trace=trace)
    LAST_RESULTS = res

    out = np.empty((B, L, D), np.float32)
    for b in range(B):
        out[b] = (res.results[2 * b]["outp"] + res.results[2 * b + 1]["outp"]
                  + wo_b[None, :])
    return out


# revision 31
# speedup vs baseline: 1.0820x; 1.0026x over previous
"""Causal MHA (B=4, L=2048, D=1024, H=16) on 8 NeuronCores.

Sharding: core c -> (batch b = c//2, head-group g = c%2). Data-parallel over
the 4 batches, tensor-parallel over heads (8 heads per core): wq/wk/wv
column-parallel, wo row-parallel. Each core returns a partial [L, D] output;
the host sums the two head-group partials per batch and adds wo_b.

Single fused streaming kernel, no DRAM round-trips:
  A(n):  projections in fp8e4m3 DoubleRow (4x PE rate). Weights pre-scaled
         x16 on host (avoids fp8 subnormals); the x16 on Q/K is undone by
         the exp() scale (2^-11 = 1/(8*256)), the x16 on V cancels against
         the x16 ones-column in the softmax ratio.
  B(n):  per head: S.T[keys,q] = KT_h.T @ QT_h (f32r), exp on ACT with
         full-history kb blocks PAIRED into [128,2,512] psum tiles (halves
         ACT per-instruction overhead) -> pt bf16; tri-mask diag tile (DVE);
         flipped AV: avps[q, 4t, 65] += pt_blk.T @ vaug (bf16, ones col =
         denominator on the partition axis) -> tensor_scalar_mul normalize.
  T(n):  ctxn [q,512] -> ctxT [d,q] via xbar DMA-transpose.
  C(n):  out[tok,1024] = sum_c ctxT[c].T @ wo[c] (bf16), interleaved into
         B(n+1) heads; A(n+1) units interleaved into B(n) to keep PE busy.
"""

import numpy as np
import ml_dtypes

import concourse.bacc as bacc
import concourse.bass as bass
import concourse.mybir as mybir
import concourse.tile as tile
from concourse.bass_utils import run_bass_kernel_spmd

F32 = mybir.dt.float32
F32R = mybir.dt.float32r
BF16 = mybir.dt.bfloat16
F8 = mybir.dt.float8e4
DR = mybir.MatmulPerfMode.DoubleRow

B, L, D, H, DK = 4, 2048, 1024, 16, 64
HD = 8             # heads per core
GW = 512           # head-group width (8 heads * 64)
AUGW = HD * (DK + 1)   # 520: per head 64 dims + ones col (ones LAST per head)
NCH = D // 128     # 8 contraction chunks
NSL = 4            # token slices of 512
NTT = L // 128     # 16 token tiles
WS = 1.0           # V-path pre-scale (cancels in softmax ratio; 1 for bf16)
ESC = 0.125        # exp scale: 1/sqrt(DK)
EBI = -2.0         # exp bias: shift-invariant headroom so exp fits fp8e4m3


def _build_nc(dbg=False):
    import os
    LOOKAHEAD = bool(int(os.environ.get("KCFG_LOOKAHEAD", "0")))
    ARATE = int(os.environ.get("KCFG_ARATE", "2"))
    CRATES = [int(x) for x in os.environ.get("KCFG_CRATE", "2,2,2,2").split(",")]
    INHEAD = bool(int(os.environ.get("KCFG_INHEAD", "0")))
    BIASACT = bool(int(os.environ.get("KCFG_BIASACT", "0")))
    WEAVE = bool(int(os.environ.get("KCFG_WEAVE", "0")))
    nc = bacc.Bacc("TRN2", target_bir_lowering=False, debug=False, num_devices=8)

    xq = nc.dram_tensor("xq", [128, NCH, L], BF16, kind="ExternalInput").ap()
    xk = nc.dram_tensor("xk", [128, NCH, L], BF16, kind="ExternalInput").ap()
    xv = nc.dram_tensor("xv", [128, NCH, L], BF16, kind="ExternalInput").ap()
    wq = nc.dram_tensor("wq", [128, NCH, GW], BF16, kind="ExternalInput").ap()
    wk = nc.dram_tensor("wk", [128, NCH, GW], BF16, kind="ExternalInput").ap()
    wv = nc.dram_tensor("wv", [128, NCH, AUGW], BF16, kind="ExternalInput").ap()
    wo = nc.dram_tensor("wo", [128, 4, D], BF16, kind="ExternalInput").ap()
    bq = nc.dram_tensor("bq", [128, 4], F32, kind="ExternalInput").ap()
    bk = nc.dram_tensor("bk", [128, 4], F32, kind="ExternalInput").ap()
    vb = nc.dram_tensor("vb", [AUGW], F32, kind="ExternalInput").ap()
    msk = nc.dram_tensor("msk", [128, 128], BF16, kind="ExternalInput").ap()
    outp = nc.dram_tensor("outp", [L, D], F32, kind="ExternalOutput").ap()
    if dbg:
        qt_dbg = nc.dram_tensor("qt_dbg", [128, 4, GW], F32,
                                kind="ExternalOutput").ap()
        kt_dbg = nc.dram_tensor("kt_dbg", [128, 4, L], F32,
                                kind="ExternalOutput").ap()
        vg_dbg = nc.dram_tensor("vg_dbg", [128, NTT, AUGW], BF16,
                                kind="ExternalOutput").ap()
        cn_dbg = nc.dram_tensor("cn_dbg", [128, NTT, GW], BF16,
                                kind="ExternalOutput").ap()
        ct_dbg = nc.dram_tensor("ct_dbg", [128, NTT, 4, 128], BF16,
                                kind="ExternalOutput").ap()

    with tile.TileContext(nc) as tc:
        with (
            tc.tile_pool(name="persist", bufs=1) as persist,
            tc.tile_pool(name="qtp", bufs=(4 if dbg else 3)) as qtp,
            tc.tile_pool(name="xqk", bufs=4) as xqkp,
            tc.tile_pool(name="xvp", bufs=3) as xvp,
            tc.tile_pool(name="ptp", bufs=(4 if dbg else 5)) as ptp,
            tc.tile_pool(name="ctxn", bufs=(16 if dbg else 6)) as ctxnp,
            tc.tile_pool(name="ctxT", bufs=16) as ctxTp,
            tc.tile_pool(name="rcp", bufs=4) as rcp,
            tc.tile_pool(name="outs", bufs=2) as outsp,
            tc.tile_pool(name="psS", bufs=2, space="PSUM") as psS,
            tc.tile_pool(name="psAV", bufs=1, space="PSUM") as psAV,
            tc.tile_pool(name="psA", bufs=2, space="PSUM") as psA,
            tc.tile_pool(name="psC", bufs=1, space="PSUM") as psC,
        ):
            # ---- persistent SBUF ----
            wq_s = persist.tile([128, NCH, GW], BF16, tag="wq")
            wk_s = persist.tile([128, NCH, GW], BF16, tag="wk")
            wv_s = persist.tile([128, NCH, AUGW], BF16, tag="wv")
            wo_s = persist.tile([128, 4, D], BF16, tag="wo")
            kt_s = persist.tile([128, 4, L], F32R, tag="kt")
            vaug_s = persist.tile([128, NTT, AUGW], BF16, tag="vaug")
            bq_s = persist.tile([128, 4], F32, tag="bq")
            bk_s = persist.tile([128, 4], F32, tag="bk")
            vb_s = persist.tile([128, AUGW], BF16, tag="vb")
            msk_s = persist.tile([128, 128], BF16, tag="msk")

            # weight/const loads; order = DMA engine order (startup latency)
            nc.sync.dma_start(bq_s[:, :], bq[:, :])
            nc.sync.dma_start(bk_s[:, :], bk[:, :])
            vb_bcast = bass.AP(tensor=vb.tensor, offset=vb.offset,
                               ap=[[0, 128], [1, AUGW]])
            nc.gpsimd.dma_start(vb_s[:, :], vb_bcast)

            xq_tiles = {}
            xk_tiles = {}
            xv_tiles = {}

            def issue_xin(n):
                c0, c1 = n * 512, (n + 1) * 512
                t = xqkp.tile([128, NCH, 512], BF16, tag="xqk", name=f"xq{n}")
                nc.sync.dma_start(t[:, :, :], xq[:, :, c0:c1])
                xq_tiles[n] = t
                t = xqkp.tile([128, NCH, 512], BF16, tag="xqk", name=f"xk{n}")
                nc.sync.dma_start(t[:, :, :], xk[:, :, c0:c1])
                xk_tiles[n] = t
                t = xvp.tile([128, NCH, 512], BF16, tag="xv", name=f"xv{n}")
                nc.sync.dma_start(t[:, :, :], xv[:, :, c0:c1])
                xv_tiles[n] = t

            # startup order matches phase-A consumption: Q, K, then V
            nc.sync.dma_start(wq_s[:, :, 0:256], wq[:, :, 0:256])
            t0 = xqkp.tile([128, NCH, 512], BF16, tag="xqk", name="xq0")
            nc.sync.dma_start(t0[:, :, 0:256], xq[:, :, 0:256])
            nc.sync.dma_start(wq_s[:, :, 256:512], wq[:, :, 256:512])
            nc.sync.dma_start(t0[:, :, 256:512], xq[:, :, 256:512])
            xq_tiles[0] = t0
            nc.sync.dma_start(wk_s[:, :, :], wk[:, :, :])
            t0 = xqkp.tile([128, NCH, 512], BF16, tag="xqk", name="xk0")
            nc.sync.dma_start(t0[:, :, :], xk[:, :, 0:512])
            xk_tiles[0] = t0
            nc.sync.dma_start(wv_s[:, :, :], wv[:, :, :])
            t0 = xvp.tile([128, NCH, 512], BF16, tag="xv", name="xv0")
            nc.sync.dma_start(t0[:, :, :], xv[:, :, 0:512])
            xv_tiles[0] = t0
            nc.sync.dma_start(msk_s[:, :], msk[:, :])
            nc.sync.dma_start(wo_s[:, :, :], wo[:, :, :])

            qt_tiles = {}

            # ---- phase A unit generator: fp8 DoubleRow projections ----
            def a_units(n):
                qt_t = qtp.tile([128, 4, GW], F32R, tag="qt", name=f"qt{n}")
                qt_tiles[n] = qt_t

                def qk_unit(hf, g, x_of, w_s, b_s, is_q):
                    # split into two ~0.9us halves (one mi each) so fillers
                    # can weave between S/exp steps without starving ACT
                    ps_box = []

                    def half(mi):
                        def emit():
                            x_t = x_of[n]
                            if mi == 0:
                                ps_box.append(psA.tile(
                                    [128, 2, 256], F32, tag="pa",
                                    name=f"pa{n}_{hf}_{g}"))
                            ps = ps_box[0]
                            for c in range(NCH):
                                # start=True zeroes the whole psum bank:
                                # only the first write into the tile sets it
                                nc.tensor.matmul(
                                    ps[:, mi, :],
                                    w_s[:, c, (2 * g + mi) * 128:
                                        (2 * g + mi + 1) * 128],
                                    x_t[:, c, hf * 256:hf * 256 + 256],
                                    start=(c == 0 and mi == 0),
                                    stop=(c == NCH - 1),
                                    skip_group_check=True)
                            m = 2 * g + mi
                            if is_q:
                                dst = qt_t[:, m, hf * 256:hf * 256 + 256]
                            else:
                                dst = kt_s[:, m, n * 512 + hf * 256:
                                           n * 512 + hf * 256 + 256]
                            if BIASACT:
                                nc.scalar.activation(
                                    dst, ps[:, mi, :],
                                    func=mybir.ActivationFunctionType.Identity,
                                    bias=b_s[:, m:m + 1])
                            else:
                                nc.vector.tensor_scalar_add(
                                    dst, ps[:, mi, :], b_s[:, m:m + 1])
                        return emit
                    return [half(0), half(1)]

                def v_unit(tt, vhf):
                    def emit():
                        ps = psA.tile([128, 260], F32, tag="pa",
                                      name=f"pv{n}_{tt}_{vhf}")
                        xv_t = xv_tiles[n]
                        for c in range(NCH):
                            nc.tensor.matmul(
                                ps[:, :],
                                xv_t[:, c, tt * 128:(tt + 1) * 128],
                                wv_s[:, c, vhf * 260:(vhf + 1) * 260],
                                start=(c == 0), stop=(c == NCH - 1))
                        nc.vector.tensor_add(
                            vaug_s[:, n * 4 + tt, vhf * 260:(vhf + 1) * 260],
                            ps[:, :], vb_s[:, vhf * 260:(vhf + 1) * 260])
                    return emit

                units = []
                for hf in range(2):
                    for g in range(2):
                        units.extend(qk_unit(hf, g, xq_tiles, wq_s, bq_s, True))
                    for g in range(2):
                        units.extend(qk_unit(hf, g, xk_tiles, wk_s, bk_s, False))
                    for tt in (2 * hf, 2 * hf + 1):
                        for vhf in range(2):
                            units.append(v_unit(tt, vhf))
                return units

            ctxn_tiles = {}
            ctxT_tiles = {}

            # ---- phase B: one head of slice n ----
            # returns (s_emitters, av_emitters, finalize) so the slice loop
            # can weave the next head's first S blocks before this head's
            # tail, keeping ACT fed across head boundaries
            def plan_head(n, h):
                po = (h % 2) * 64
                mc = h // 2
                qt_t = qt_tiles[n]
                nkb = 4 * n + 4
                avps = psAV.tile([128, 4, DK + 1], F32, tag="av",
                                 name=f"av{n}_{h}")
                # S/exp units: full-history kb pairs, then 4 single diag blocks
                pt_of = {}   # kb -> (tile, region or None)
                sunits = [("pair", p) for p in range(2 * n)]
                sunits += [("diag", kb) for kb in range(4 * n, 4 * n + 4)]

                def emit_s(u):
                    kind, a = u
                    sp = psS.tile([128, 2, 512], F32, tag="sp",
                                  name=f"sp{n}_{h}_{kind}{a}")
                    if kind == "pair":
                        for i in range(2):
                            kb = 2 * a + i
                            # regions 0/1 are in different banks: each needs
                            # its own start=True (bank-granular zeroing)
                            nc.tensor.matmul(
                                sp[:, i, :],
                                kt_s[po:po + 64, mc, kb * 128:(kb + 1) * 128],
                                qt_t[po:po + 64, mc, :],
                                start=True, stop=True,
                                skip_group_check=True)
                        pt = ptp.tile([128, 2, 512], BF16, tag="pt",
                                      name=f"pt{n}_{h}_p{a}")
                        nc.scalar.activation(
                            pt[:, :, :], sp[:, :, :],
                            func=mybir.ActivationFunctionType.Exp, scale=ESC)
                        pt_of[2 * a] = (pt, 0)
                        pt_of[2 * a + 1] = (pt, 1)
                    else:
                        kb = a
                        jj = kb - 4 * n
                        col0e = jj * 128
                        col0s = min(col0e, 256)
                        nc.tensor.matmul(
                            sp[:, 0, col0s:],
                            kt_s[po:po + 64, mc, kb * 128:(kb + 1) * 128],
                            qt_t[po:po + 64, mc, col0s:],
                            start=True, stop=True, skip_group_check=True)
                        pt = ptp.tile([128, 512], BF16, tag="pt",
                                      name=f"pt{n}_{h}_d{jj}")
                        nc.scalar.activation(
                            pt[:, col0e:], sp[:, 0, col0e:],
                            func=mybir.ActivationFunctionType.Exp, scale=ESC)
                        nc.vector.tensor_mul(
                            pt[:, col0e:col0e + 128],
                            pt[:, col0e:col0e + 128], msk_s[:, :])
                        pt_of[kb] = (pt, None)

                def emit_av(u):
                    kind, a = u
                    kbs = [2 * a, 2 * a + 1] if kind == "pair" else [a]
                    for kb in kbs:
                        j0 = max(0, kb - 4 * n)
                        pt, reg = pt_of[kb]
                        for j in range(j0, 4):
                            lhs = (pt[:, reg, j * 128:(j + 1) * 128]
                                   if reg is not None
                                   else pt[:, j * 128:(j + 1) * 128])
                            # whole-bank zero on start: only first mm sets it
                            nc.tensor.matmul(
                                avps[:, j, :], lhs,
                                vaug_s[:, kb, h * 65:(h + 1) * 65],
                                start=(kb == 0 and j == 0),
                                stop=(kb == 4 * n + j),
                                skip_group_check=True)

                def finalize():
                    rc = rcp.tile([128, 4], F32, tag="rc", name=f"rc{n}_{h}")
                    nc.vector.reciprocal(rc[:, :], avps[:, :, 64])
                    for j in range(4):
                        nc.vector.tensor_scalar_mul(
                            ctxn_tiles[(n, j)][:, h * 64:(h + 1) * 64],
                            avps[:, j, 0:64], rc[:, j:j + 1])

                s_emit = [(lambda u: (lambda: emit_s(u)))(u) for u in sunits]
                av_emit = [(lambda u: (lambda: emit_av(u)))(u) for u in sunits]
                return s_emit, av_emit, finalize

            # ---- phase C unit: token tile t, output half n2 ----
            out_tiles = {}

            def c_unit(n, j, n2):
                t = 4 * n + j
                # all C units run during B(3)/tail where psA is idle:
                # alternate psC/psA slots for deeper pipelining
                pool = psA if (2 * j + n2) % 2 == 1 else psC
                ptag = "pa" if pool is psA else "cps"

                def emit():
                    if n2 == 0:
                        out_tiles[t] = outsp.tile([128, D], F32, tag="outs",
                                                  name=f"out{t}")
                    cps = pool.tile([128, 512], F32, tag=ptag,
                                    name=f"cps{t}_{n2}")
                    ctxT_t = ctxT_tiles[(n, j)]
                    for c in range(4):
                        nc.tensor.matmul(
                            cps[:, :], ctxT_t[:, c, :],
                            wo_s[:, c, n2 * 512:(n2 + 1) * 512],
                            start=(c == 0), stop=(c == 3))
                    nc.vector.tensor_copy(
                        out_tiles[t][:, n2 * 512:(n2 + 1) * 512], cps[:, :])
                    if n2 == 1:
                        nc.sync.dma_start(
                            outp[t * 128:(t + 1) * 128, :], out_tiles[t][:, :])
                return emit

            # ---- main schedule ----
            for u in a_units(0):
                u()

            a_queue = []          # (slice, unit) in slice order
            pending_c = []
            c_rate = dict(enumerate(CRATES))

            for n in range(NSL):
                if LOOKAHEAD:
                    if n == 0:
                        issue_xin(1)
                        issue_xin(2)
                        for u in a_units(1):
                            a_queue.append((1, u))
                        for u in a_units(2):
                            a_queue.append((2, u))
                    elif n == 1:
                        issue_xin(3)
                        for u in a_units(3):
                            a_queue.append((3, u))
                else:
                    if n < NSL - 1:
                        issue_xin(n + 1)
                        for u in a_units(n + 1):
                            a_queue.append((n + 1, u))
                for j in range(4):
                    ctxn_tiles[(n, j)] = ctxnp.tile(
                        [128, GW], BF16, tag="ctxn", name=f"ctxn{n}_{j}")
                fillq = []
                for _ in range(ARATE * HD):
                    if a_queue:
                        fillq.append(a_queue.pop(0)[1])
                for _ in range(c_rate.get(n, 2) * HD):
                    if pending_c:
                        fillq.append(pending_c.pop(0))
                if WEAVE:
                    prev_fin = None
                    for h in range(HD):
                        s_emit, av_emit, fin = plan_head(n, h)
                        ns_ = len(s_emit)
                        s_emit[0]()
                        if prev_fin is not None:
                            prev_fin()
                        s_emit[1]()
                        if fillq:
                            fillq.pop(0)()
                        for i in range(2, ns_):
                            s_emit[i]()
                            av_emit[i - 2]()
                            if fillq:
                                fillq.pop(0)()
                        av_emit[ns_ - 2]()
                        av_emit[ns_ - 1]()
                        prev_fin = fin
                    prev_fin()
                    while fillq:
                        fillq.pop(0)()
                else:
                    nfill = len(fillq)
                    for h in range(HD):
                        s_emit, av_emit, fin = plan_head(n, h)
                        ns_ = len(s_emit)
                        s_emit[0]()
                        if ns_ > 1:
                            s_emit[1]()
                        for i in range(2, ns_):
                            s_emit[i]()
                            av_emit[i - 2]()
                        av_emit[ns_ - 2]()
                        av_emit[ns_ - 1]()
                        fin()
                        take = (nfill * (h + 1)) // HD - (nfill * h) // HD
                        for _ in range(take):
                            if fillq:
                                fillq.pop(0)()
                # B(n+1) needs all of A(n+1) done
                while a_queue and a_queue[0][0] <= n + 1:
                    a_queue.pop(0)[1]()
                for j in range(4):
                    ct = ctxTp.tile([128, 4, 128], BF16, tag="ctxT",
                                    name=f"ctxT{n}_{j}")
                    nc.sync.dma_start_transpose(ct, ctxn_tiles[(n, j)][:, :])
                    ctxT_tiles[(n, j)] = ct
                for j in range(4):
                    for n2 in range(2):
                        pending_c.append(c_unit(n, j, n2))
            while pending_c:
                pending_c.pop(0)()

            if dbg:
                nc.sync.dma_start(qt_dbg[:, :, :],
                                  qt_tiles[0][:, :, :].bitcast(F32))
                nc.sync.dma_start(kt_dbg[:, :, :], kt_s[:, :, :].bitcast(F32))
                nc.sync.dma_start(vg_dbg[:, :, :], vaug_s[:, :, :])
                for n in range(NSL):
                    for j in range(4):
                        nc.sync.dma_start(cn_dbg[:, 4 * n + j, :],
                                          ctxn_tiles[(n, j)][:, :])
                        nc.sync.dma_start(ct_dbg[:, 4 * n + j, :, :],
                                          ctxT_tiles[(n, j)][:, :, :])

    nc.compile()
    return nc


_NC = None
LAST_RESULTS = None


def kernel(**inputs):
    global _NC, LAST_RESULTS
    import os
    if _NC is None:
        _NC = _build_nc()

    f = lambda a: np.asarray(a, dtype=np.float32)
    q, k, v = f(inputs["q"]), f(inputs["k"]), f(inputs["v"])
    wq_w, wq_b = f(inputs["wq_w"]), f(inputs["wq_b"])
    wk_w, wk_b = f(inputs["wk_w"]), f(inputs["wk_b"])
    wv_w, wv_b = f(inputs["wv_w"]), f(inputs["wv_b"])
    wo_w, wo_b = f(inputs["wo_w"]), f(inputs["wo_b"])

    bf = ml_dtypes.bfloat16
    f8 = ml_dtypes.float8_e4m3

    def chunk_rows(a, inner):
        # [1024, X] -> [128, 8, X] with row r = c*128+p -> [p, c, :]
        return np.ascontiguousarray(
            a.reshape(NCH, 128, inner).transpose(1, 0, 2))

    msk = np.ascontiguousarray(
        (np.arange(128)[None, :] >= np.arange(128)[:, None])).astype(bf)

    gmaps = []
    for g in range(2):
        sl = slice(g * GW, (g + 1) * GW)
        wqT = chunk_rows(wq_w[sl].T, GW).astype(bf)
        wkT = chunk_rows(wk_w[sl].T, GW).astype(bf)
        wvT = np.zeros((D, AUGW), np.float32)
        vbias = np.zeros((AUGW,), np.float32)
        for h in range(HD):
            wvT[:, h * 65:h * 65 + 64] = wv_w[g * GW + h * 64:
                                              g * GW + (h + 1) * 64].T * WS
            vbias[h * 65:h * 65 + 64] = wv_b[g * GW + h * 64:
                                             g * GW + (h + 1) * 64] * WS
            vbias[h * 65 + 64] = WS
        woT = np.ascontiguousarray(
            wo_w[:, sl].T.reshape(4, 128, D).transpose(1, 0, 2)).astype(bf)
        bqT = np.ascontiguousarray(wq_b[sl].reshape(4, 128).T)
        bkT = np.ascontiguousarray(wk_b[sl].reshape(4, 128).T)
        gmaps.append(dict(wq=wqT, wk=wkT, wv=chunk_rows(wvT, AUGW).astype(bf),
                          wo=woT, bq=bqT, bk=bkT, vb=vbias, msk=msk))

    bmaps = []
    for b in range(B):
        bmaps.append(dict(
            xq=chunk_rows(np.ascontiguousarray(q[b].T), L).astype(bf),
            xk=chunk_rows(np.ascontiguousarray(k[b].T), L).astype(bf),
            xv=chunk_rows(np.ascontiguousarray(v[b].T), L).astype(bf)))

    in_maps = [dict(**bmaps[c // 2], **gmaps[c % 2]) for c in range(8)]

    trace = bool(int(os.environ.get("KERNEL_TRACE", "0")))
    res = run_bass_kernel_spmd(_NC, in_maps, list(range(8)), trace=trace)
    LAST_RESULTS = res

    out = np.empty((B, L, D), np.float32)
    for b in range(B):
        out[b] = (res.results[2 * b]["outp"] + res.results[2 * b + 1]["outp"]
                  + wo_b[None, :])
    return out
